# revision 20
# baseline (speedup 1.0000x reference)
"""Distributed Trainium2 kernel for nn_Criterion_35012573397697.

Proxy-NCA-style loss: mean_b[ d(x_b, p_{y_b}) + logsumexp_{c != y_b}(-d(x_b, p_c)) ]
with x = 3*l2norm(batch), p = 3*l2norm(proxies), d = squared euclidean.

Strategy (8 NeuronCores, classes sharded):
  - d(x,p) = 18 - 2*x.p (both live on the 3-sphere), so only the dot products
    are needed. Each core owns C/8 = 12500 classes (zero-padded to 12800).
  - Per core: normalize its proxy shard (norms via GpSimd square + DVE reduce,
    scale folded into the f32->bf16 weight conversion), transpose via the PE,
    then 100 bf16 matmuls [b=128,c=512] against the transposed normalized
    batch. ScalarE computes exp(2*s - 18) with a fused row-sum (accum_out)
    directly from PSUM -> per-b partial sums over the local classes.
  - One small AllGather of the [512] partial sums; every core reduces the 8
    partials and computes the final scalar identically (core 0's output is
    returned). The positive-class distance is computed exactly in f32 from
    host-gathered proxies[labels] rows (indexing only, no host arithmetic),
    and exp(-pos_dist) plus the exact zero-pad contribution are subtracted
    from the gathered sum before the log.
"""

import math

import numpy as np
import ml_dtypes

import concourse.bass as bass
import concourse.bacc as bacc
import concourse.mybir as mybir
import concourse.tile as tile
from concourse.bass_utils import run_bass_kernel_spmd

N_CORES = 8
B = 512
D = 128
C = 100000
SH = 12800           # padded shard size per core
NT = SH // 128       # 100 c-tiles of 128
BT = B // 128        # 4 b-tiles
PAD_ROWS = N_CORES * SH - C   # 2400 zero rows in total
PAD_CORR = PAD_ROWS * math.exp(-18.0)

F32 = mybir.dt.float32
BF16 = mybir.dt.bfloat16
AX = mybir.AxisListType
OP = mybir.AluOpType
AF = mybir.ActivationFunctionType

_CACHE = {}


def _rsqrt_dve(nc, pool, F32, OP, dst, src, n, scale=1.0):
    """dst = scale / sqrt(src) via Quake III bit trick + one Newton step (DVE only)."""
    I32 = mybir.dt.int32
    v = pool.tile([128, n], F32, tag="rsq_v")
    nc.vector.tensor_scalar(v[:], src, 1e-12, None, OP.max)
    src = v[:]
    h = pool.tile([128, n], I32, tag="rsq_h")
    nc.vector.tensor_scalar(h[:], src.bitcast(I32), 1, None,
                            OP.logical_shift_right)
    y0 = pool.tile([128, n], I32, tag="rsq_y0")
    nc.vector.tensor_scalar(y0[:], h[:], -1, 0x5F3759DF, OP.mult, OP.add)
    y0f = y0[:].bitcast(F32)
    t = pool.tile([128, n], F32, tag="rsq_t")
    y1 = pool.tile([128, n], F32, tag="rsq_y1")
    nc.vector.tensor_tensor(t[:], y0f, y0f, OP.mult)        # y0^2
    nc.vector.tensor_tensor(t[:], t[:], src, OP.mult)       # v*y0^2
    nc.vector.tensor_scalar(t[:], t[:], -0.5, 1.5, OP.mult, OP.add)
    nc.vector.tensor_tensor(y1[:], y0f, t[:], OP.mult)      # Newton 1
    nc.vector.tensor_tensor(t[:], y1[:], y1[:], OP.mult)    # y1^2
    nc.vector.tensor_tensor(t[:], t[:], src, OP.mult)       # v*y1^2
    nc.vector.tensor_scalar(t[:], t[:], -0.5 * scale, 1.5 * scale,
                            OP.mult, OP.add)
    nc.vector.tensor_tensor(dst, y1[:], t[:], OP.mult)      # Newton 2



def build_graph(stage=9):
    nc = bacc.Bacc("TRN2", target_bir_lowering=False, debug=False,
                   num_devices=N_CORES)
    p_ext = nc.dram_tensor("pshard", [SH, D], F32, kind="ExternalInput").ap()
    b_ext = nc.dram_tensor("batch", [B, D], F32, kind="ExternalInput").ap()
    sel_ext = nc.dram_tensor("psel", [B, D], F32, kind="ExternalInput").ap()
    id_ext = nc.dram_tensor("ident", [128, 128], BF16, kind="ExternalInput").ap()
    out_ext = nc.dram_tensor("out", [1, 1], F32, kind="ExternalOutput").ap()

    with tile.TileContext(nc) as tc:
        with tc.tile_pool(name="dram", bufs=1, space="DRAM") as dram, \
             tc.tile_pool(name="big", bufs=1) as bigp, \
             tc.tile_pool(name="sb", bufs=2) as pool, \
             tc.tile_pool(name="ps", bufs=2, space="PSUM") as psp:

            # ---------- loads ----------
            praw = bigp.tile([128, NT, 128], F32)       # [c%128, t, d]
            psrc = p_ext.rearrange("(t p) d -> p t d", p=128)
            for i in range(4):
                k = NT // 4
                nc.sync.dma_start(praw[:, i * k:(i + 1) * k, :],
                                  psrc[:, i * k:(i + 1) * k, :])
            xb = bigp.tile([128, BT, 128], F32)         # [b%128, bt, d]
            nc.sync.dma_start(xb[:], b_ext.rearrange("(t p) d -> p t d", p=128))
            selb = bigp.tile([128, BT, 128], F32)
            nc.sync.dma_start(selb[:], sel_ext.rearrange("(t p) d -> p t d", p=128))
            ident = bigp.tile([128, 128], BF16)
            nc.sync.dma_start(ident[:], id_ext[:])

            # early tiny AllGather: absorbs the first-collective warm-up
            # while the real compute runs; result folded in as exact zero.
            DUMMY_AG = True
            dag_in = dram.tile([1, 16], F32)
            dag_out = dram.tile([N_CORES, 16], F32)
            z16 = bigp.tile([1, 16], F32)
            nc.vector.memset(z16[:], 0.0)
            dagj = bigp.tile([1, 1], F32)
            if DUMMY_AG:
                nc.sync.dma_start(dag_in[:], z16[:])
                nc.gpsimd.collective_compute(
                    "AllGather", OP.bypass,
                    replica_groups=[list(range(N_CORES))],
                    ins=[dag_in.opt()], outs=[dag_out.opt()],
                )
                nc.sync.dma_start(dagj[:], dag_out[0:1, 0:1])
            else:
                nc.vector.memset(dagj[:], 0.0)

            eps_t = bigp.tile([128, 1], F32)
            nc.vector.memset(eps_t[:], 1e-24)
            bias18 = bigp.tile([128, 1], F32)
            nc.vector.memset(bias18[:], -18.0)

            # ---------- x / psel prep (tiny) ----------
            n2 = bigp.tile([128, 2 * BT], F32)
            sqx = pool.tile([128, BT, 128], F32, tag="sqscr")
            nc.vector.tensor_tensor(sqx[:], xb[:], xb[:], OP.mult)
            nc.vector.tensor_reduce(n2[:, 0:BT], sqx[:], axis=AX.X, op=OP.add)
            sqs = pool.tile([128, BT, 128], F32, tag="sqscr")
            nc.vector.tensor_tensor(sqs[:], selb[:], selb[:], OP.mult)
            nc.vector.tensor_reduce(n2[:, BT:2 * BT], sqs[:], axis=AX.X,
                                    op=OP.add)
            rn = bigp.tile([128, 2 * BT], F32)   # 1/sqrt(n2)
            _rsqrt_dve(nc, pool, F32, OP, rn[:], n2[:], 2 * BT)

            posdot = bigp.tile([128, BT], F32)
            sqd = pool.tile([128, BT, 128], F32, tag="sqscr")
            nc.vector.tensor_tensor(sqd[:], xb[:], selb[:], OP.mult)
            nc.vector.tensor_reduce(posdot[:], sqd[:], axis=AX.X, op=OP.add)
            posd = bigp.tile([128, BT], F32)
            tmp4 = pool.tile([128, BT], F32, tag="smallscr")
            nc.vector.tensor_tensor(tmp4[:], posdot[:], rn[:, 0:BT], OP.mult)
            nc.vector.tensor_tensor(tmp4[:], tmp4[:], rn[:, BT:2 * BT], OP.mult)
            nc.vector.tensor_scalar(posd[:], tmp4[:], -18.0, 18.0, OP.mult,
                                    OP.add)

            if stage >= 2:
                xscale3 = bigp.tile([128, BT], F32)
                nc.vector.tensor_scalar_mul(xscale3[:], rn[:, 0:BT], 3.0)
                xhat = bigp.tile([128, BT, 128], BF16)
                for t in range(BT):
                    nc.vector.tensor_scalar_mul(xhat[:, t, :], xb[:, t, :],
                                                xscale3[:, t:t + 1])
                xT = bigp.tile([128, BT, 128], BF16)
                xps = psp.tile([128, 8 * 128], BF16, tag="ps")
                for t in range(BT):
                    nc.tensor.transpose(xps[:, t * 128:(t + 1) * 128],
                                        xhat[:, t, :], ident[:])
                nc.vector.tensor_copy(
                    xT[:], xps[:, 0:BT * 128].rearrange("p (t d) -> p t d", t=BT))

            if stage >= 3:
                psq = bigp.tile([128, NT, 128], BF16)
                pn2 = bigp.tile([128, NT], F32)
                pscale3 = bigp.tile([128, NT], F32)
                pbf = bigp.tile([128, NT, 128], BF16)
                pT = bigp.tile([128, NT, 128], BF16)   # [d, t, c%128]
                NG = 7
                partials = bigp.tile([128, BT * NG], F32)
                pTf = pT[:].rearrange("p t c -> p (t c)")

                def issue_group(bt, g):
                    w = 4 if g < 6 else 1
                    sp = psp.tile([128, 2048], F32, tag="ps")
                    for j in range(w):
                        ch = g * 4 + j
                        nc.tensor.matmul(
                            sp[:, j * 512:(j + 1) * 512],
                            xT[:, bt, :],
                            pTf[:, ch * 512:(ch + 1) * 512],
                            start=True, stop=True)
                    ej = pool.tile([128, 2048], BF16, tag="ejunk")
                    nc.scalar.activation(
                        ej[:, 0:w * 512], sp[:, 0:w * 512], AF.Exp,
                        bias=bias18[:, 0:1], scale=2.0,
                        accum_out=partials[:, bt * NG + g:bt * NG + g + 1])

                # groups issued once their pT chunks are drained:
                # after chunk i, complete 512-chunks = floor(25*(i+1)/4)
                sched = {0: [(bt, 0) for bt in range(BT)],
                         1: [(bt, g) for g in (1, 2) for bt in range(BT)],
                         2: [(bt, 3) for bt in range(BT)],
                         3: [(bt, g) for g in (4, 5, 6) for bt in range(BT)]}

                k = NT // 4
                for i in range(4):
                    lo, hi = i * k, (i + 1) * k
                    nc.gpsimd.tensor_tensor(psq[:, lo:hi, :],
                                            praw[:, lo:hi, :],
                                            praw[:, lo:hi, :], OP.mult)
                    nc.vector.tensor_reduce(pn2[:, lo:hi], psq[:, lo:hi, :],
                                            axis=AX.X, op=OP.add)
                    _rsqrt_dve(nc, pool, F32, OP, pscale3[:, lo:hi],
                               pn2[:, lo:hi], k, scale=3.0)
                    if stage >= 4:
                        for g0 in range(lo, hi, 8):
                            w = min(8, hi - g0)
                            nc.vector.tensor_tensor(
                                pbf[:, g0:g0 + w, :], praw[:, g0:g0 + w, :],
                                pscale3[:, g0:g0 + w, None].to_broadcast(
                                    (128, w, 128)),
                                OP.mult)
                            tp = psp.tile([128, 8 * 128], BF16, tag="ps")
                            for j in range(w):
                                nc.tensor.transpose(
                                    tp[:, j * 128:(j + 1) * 128],
                                    pbf[:, g0 + j, :], ident[:])
                            nc.vector.tensor_copy(
                                pT[:, g0:g0 + w, :],
                                tp[:, 0:w * 128].rearrange(
                                    "p (t d) -> p t d", t=w))
                    if stage >= 5:
                        for bt, g in sched[i]:
                            issue_group(bt, g)

            if stage >= 5:
                s_loc = bigp.tile([128, BT], F32)
                nc.vector.tensor_reduce(
                    s_loc[:], partials[:].rearrange("p (t g) -> p t g", t=BT),
                    axis=AX.X, op=OP.add)

            if stage >= 6:
                ag_in = dram.tile([128, BT], F32)
                ag_out = dram.tile([128 * N_CORES, BT], F32)
                nc.sync.dma_start(ag_in[:], s_loc[:])
                nc.gpsimd.collective_compute(
                    "AllGather", OP.bypass,
                    replica_groups=[list(range(N_CORES))],
                    ins=[ag_in.opt()], outs=[ag_out.opt()],
                )
                gath = bigp.tile([128, BT, N_CORES], F32)
                nc.sync.dma_start(gath[:],
                                  ag_out.rearrange("(r p) f -> p f r", p=128))
                s_tot = bigp.tile([128, BT], F32)
                nc.vector.tensor_reduce(s_tot[:], gath[:], axis=AX.X,
                                        op=OP.add)

            if stage >= 7:
                npos = pool.tile([128, BT], F32, tag="fin")
                nc.scalar.activation(npos[:], posd[:], AF.Exp, scale=-1.0)
                s1 = pool.tile([128, BT], F32, tag="fin")
                nc.vector.tensor_scalar(s1[:], s_tot[:], -float(PAD_CORR),
                                        None, OP.add)
                nc.vector.tensor_tensor(s1[:], s1[:], npos[:], OP.subtract)
                lse = pool.tile([128, BT], F32, tag="fin")
                nc.scalar.activation(lse[:], s1[:], AF.Ln)
                perb = pool.tile([128, BT], F32, tag="fin")
                nc.vector.tensor_tensor(perb[:], posd[:], lse[:], OP.add)
                csum = pool.tile([128, 1], F32, tag="fin")
                nc.vector.tensor_reduce(csum[:], perb[:], axis=AX.X, op=OP.add)
                nc.vector.tensor_tensor(csum[0:1, 0:1], csum[0:1, 0:1],
                                        dagj[:], OP.add)
                ones = pool.tile([128, 1], F32, tag="fin")
                nc.vector.memset(ones[:], 1.0)
                lps = psp.tile([1, 1], F32, tag="ps")
                nc.tensor.matmul(lps[:], ones[:], csum[:], start=True,
                                 stop=True)
                res = pool.tile([1, 1], F32, tag="fin")
                nc.scalar.activation(res[:], lps[:], AF.Copy, scale=1.0 / B)
                nc.sync.dma_start(out_ext[:], res[:])
            elif stage == 1:
                nc.sync.dma_start(out_ext[:], posd[0:1, 0:1])
            elif stage == 2:
                nc.gpsimd.dma_start(out_ext[:], xT[0:1, 0, 0:1])
            elif stage == 3:
                nc.sync.dma_start(out_ext[:], pscale3[0:1, 0:1])
            elif stage == 4:
                nc.gpsimd.dma_start(out_ext[:], pT[0:1, 0, 0:1])
            elif stage == 5:
                nc.sync.dma_start(out_ext[:], s_loc[0:1, 0:1])
            elif stage == 6:
                nc.sync.dma_start(out_ext[:], s_tot[0:1, 0:1])

    nc.compile()
    return nc


def make_in_maps(batch, labels, proxies):
    batch = np.ascontiguousarray(batch, dtype=np.float32)
    labels = np.asarray(labels).astype(np.int64)
    proxies = np.ascontiguousarray(proxies, dtype=np.float32)
    psel = np.ascontiguousarray(proxies[labels])        # indexing only
    ident = np.eye(128, dtype=np.float32).astype(ml_dtypes.bfloat16)
    ppad = np.zeros((N_CORES * SH, D), dtype=np.float32)
    ppad[:C] = proxies
    in_maps = []
    for i in range(N_CORES):
        in_maps.append({
            "pshard": np.ascontiguousarray(ppad[i * SH:(i + 1) * SH]),
            "batch": batch,
            "psel": psel,
            "ident": ident,
        })
    return in_maps


def _get_nc():
    if "nc" not in _CACHE:
        _CACHE["nc"] = build_graph()
    return _CACHE["nc"]


def kernel(batch, labels, proxies):
    nc = _get_nc()
    in_maps = make_in_maps(batch, labels, proxies)
    res = run_bass_kernel_spmd(nc, in_maps, core_ids=list(range(N_CORES)))
    return np.float32(res.results[0]["out"][0, 0])


if __name__ == "__main__":
    rng = np.random.default_rng(0)
    batch = rng.standard_normal((B, D)).astype(np.float32)
    labels = rng.integers(0, C, B).astype(np.int64)
    proxies = (rng.standard_normal((C, D)).astype(np.float32) / 8)
    out = kernel(batch=batch, labels=labels, proxies=proxies)
    print("loss:", out)


# revision 21
# speedup vs baseline: 1.1114x; 1.1114x over previous
"""Distributed Trainium2 kernel for nn_Criterion_35012573397697.

Proxy-NCA-style loss: mean_b[ d(x_b, p_{y_b}) + logsumexp_{c != y_b}(-d(x_b, p_c)) ]
with x = 3*l2norm(batch), p = 3*l2norm(proxies), d = squared euclidean.

Strategy (8 NeuronCores, classes sharded):
  - d(x,p) = 18 - 2*x.p (both live on the 3-sphere), so only the dot products
    are needed. Each core owns C/8 = 12500 classes (zero-padded to 12800).
  - Per core: normalize its proxy shard (norms via GpSimd square + DVE reduce,
    scale folded into the f32->bf16 weight conversion), transpose via the PE,
    then 100 bf16 matmuls [b=128,c=512] against the transposed normalized
    batch. ScalarE computes exp(2*s - 18) with a fused row-sum (accum_out)
    directly from PSUM -> per-b partial sums over the local classes.
  - One small AllGather of the [512] partial sums; every core reduces the 8
    partials and computes the final scalar identically (core 0's output is
    returned). The positive-class distance is computed exactly in f32 from
    host-gathered proxies[labels] rows (indexing only, no host arithmetic),
    and exp(-pos_dist) plus the exact zero-pad contribution are subtracted
    from the gathered sum before the log.
"""

import math

import numpy as np
import ml_dtypes

import concourse.bass as bass
import concourse.bacc as bacc
import concourse.mybir as mybir
import concourse.tile as tile
from concourse.bass_utils import run_bass_kernel_spmd

N_CORES = 8
B = 512
D = 128
C = 100000
SH = 12800           # padded shard size per core
NT = SH // 128       # 100 c-tiles of 128
BT = B // 128        # 4 b-tiles
PAD_ROWS = N_CORES * SH - C   # 2400 zero rows in total
PAD_CORR = PAD_ROWS * math.exp(-18.0)

F32 = mybir.dt.float32
BF16 = mybir.dt.bfloat16
AX = mybir.AxisListType
OP = mybir.AluOpType
AF = mybir.ActivationFunctionType

_CACHE = {}


def _rsqrt_dve(nc, pool, F32, OP, dst, src, n, scale=1.0):
    """dst = scale / sqrt(src) via Quake III bit trick + one Newton step (DVE only)."""
    I32 = mybir.dt.int32
    v = pool.tile([128, n], F32, tag="rsq_v")
    nc.vector.tensor_scalar(v[:], src, 1e-12, None, OP.max)
    src = v[:]
    h = pool.tile([128, n], I32, tag="rsq_h")
    nc.vector.tensor_scalar(h[:], src.bitcast(I32), 1, None,
                            OP.logical_shift_right)
    y0 = pool.tile([128, n], I32, tag="rsq_y0")
    nc.vector.tensor_scalar(y0[:], h[:], -1, 0x5F3759DF, OP.mult, OP.add)
    y0f = y0[:].bitcast(F32)
    t = pool.tile([128, n], F32, tag="rsq_t")
    y1 = pool.tile([128, n], F32, tag="rsq_y1")
    nc.vector.tensor_tensor(t[:], y0f, y0f, OP.mult)        # y0^2
    nc.vector.tensor_tensor(t[:], t[:], src, OP.mult)       # v*y0^2
    nc.vector.tensor_scalar(t[:], t[:], -0.5, 1.5, OP.mult, OP.add)
    nc.vector.tensor_tensor(y1[:], y0f, t[:], OP.mult)      # Newton 1
    nc.vector.tensor_tensor(t[:], y1[:], y1[:], OP.mult)    # y1^2
    nc.vector.tensor_tensor(t[:], t[:], src, OP.mult)       # v*y1^2
    nc.vector.tensor_scalar(t[:], t[:], -0.5 * scale, 1.5 * scale,
                            OP.mult, OP.add)
    nc.vector.tensor_tensor(dst, y1[:], t[:], OP.mult)      # Newton 2



def build_graph(stage=9):
    nc = bacc.Bacc("TRN2", target_bir_lowering=False, debug=False,
                   num_devices=N_CORES)
    p_ext = nc.dram_tensor("pshard", [SH, D], F32, kind="ExternalInput").ap()
    b_ext = nc.dram_tensor("batch", [B, D], F32, kind="ExternalInput").ap()
    sel_ext = nc.dram_tensor("psel", [B, D], F32, kind="ExternalInput").ap()
    id_ext = nc.dram_tensor("ident", [128, 128], BF16, kind="ExternalInput").ap()
    out_ext = nc.dram_tensor("out", [1, 1], F32, kind="ExternalOutput").ap()

    with tile.TileContext(nc) as tc:
        with tc.tile_pool(name="dram", bufs=1, space="DRAM") as dram, \
             tc.tile_pool(name="big", bufs=1) as bigp, \
             tc.tile_pool(name="sb", bufs=2) as pool, \
             tc.tile_pool(name="ps", bufs=2, space="PSUM") as psp:

            # ---------- loads ----------
            CHUNKS = [16, 28, 28, 28]
            praw = bigp.tile([128, NT, 128], F32)       # [c%128, t, d]
            psrc = p_ext.rearrange("(t p) d -> p t d", p=128)
            clo = 0
            for ck in CHUNKS:
                nc.sync.dma_start(praw[:, clo:clo + ck, :],
                                  psrc[:, clo:clo + ck, :])
                clo += ck
            xb = bigp.tile([128, BT, 128], F32)         # [b%128, bt, d]
            nc.sync.dma_start(xb[:], b_ext.rearrange("(t p) d -> p t d", p=128))
            selb = bigp.tile([128, BT, 128], F32)
            nc.sync.dma_start(selb[:], sel_ext.rearrange("(t p) d -> p t d", p=128))
            ident = bigp.tile([128, 128], BF16)
            nc.sync.dma_start(ident[:], id_ext[:])

            # early tiny AllGather: absorbs the first-collective warm-up
            # while the real compute runs; result folded in as exact zero.
            DUMMY_AG = True
            dag_in = dram.tile([1, 16], F32)
            dag_out = dram.tile([N_CORES, 16], F32)
            z16 = bigp.tile([1, 16], F32)
            nc.vector.memset(z16[:], 0.0)
            dagj = bigp.tile([1, 1], F32)
            if DUMMY_AG:
                nc.sync.dma_start(dag_in[:], z16[:])
                nc.gpsimd.collective_compute(
                    "AllGather", OP.bypass,
                    replica_groups=[list(range(N_CORES))],
                    ins=[dag_in.opt()], outs=[dag_out.opt()],
                )
                nc.sync.dma_start(dagj[:], dag_out[0:1, 0:1])
            else:
                nc.vector.memset(dagj[:], 0.0)

            eps_t = bigp.tile([128, 1], F32)
            nc.vector.memset(eps_t[:], 1e-24)
            bias18 = bigp.tile([128, 1], F32)
            nc.vector.memset(bias18[:], -18.0)

            # ---------- x / psel prep (tiny) ----------
            n2 = bigp.tile([128, 2 * BT], F32)
            sqx = pool.tile([128, BT, 128], F32, tag="sqscr")
            nc.vector.tensor_tensor(sqx[:], xb[:], xb[:], OP.mult)
            nc.vector.tensor_reduce(n2[:, 0:BT], sqx[:], axis=AX.X, op=OP.add)
            sqs = pool.tile([128, BT, 128], F32, tag="sqscr")
            nc.vector.tensor_tensor(sqs[:], selb[:], selb[:], OP.mult)
            nc.vector.tensor_reduce(n2[:, BT:2 * BT], sqs[:], axis=AX.X,
                                    op=OP.add)
            rn = bigp.tile([128, 2 * BT], F32)   # 1/sqrt(n2)
            _rsqrt_dve(nc, pool, F32, OP, rn[:], n2[:], 2 * BT)

            posdot = bigp.tile([128, BT], F32)
            sqd = pool.tile([128, BT, 128], F32, tag="sqscr")
            nc.vector.tensor_tensor(sqd[:], xb[:], selb[:], OP.mult)
            nc.vector.tensor_reduce(posdot[:], sqd[:], axis=AX.X, op=OP.add)
            posd = bigp.tile([128, BT], F32)
            tmp4 = pool.tile([128, BT], F32, tag="smallscr")
            nc.vector.tensor_tensor(tmp4[:], posdot[:], rn[:, 0:BT], OP.mult)
            nc.vector.tensor_tensor(tmp4[:], tmp4[:], rn[:, BT:2 * BT], OP.mult)
            nc.vector.tensor_scalar(posd[:], tmp4[:], -18.0, 18.0, OP.mult,
                                    OP.add)

            if stage >= 2:
                xscale3 = bigp.tile([128, BT], F32)
                nc.vector.tensor_scalar_mul(xscale3[:], rn[:, 0:BT], 3.0)
                xhat = bigp.tile([128, BT, 128], BF16)
                for t in range(BT):
                    nc.vector.tensor_scalar_mul(xhat[:, t, :], xb[:, t, :],
                                                xscale3[:, t:t + 1])
                xT = bigp.tile([128, BT, 128], BF16)
                xps = psp.tile([128, 8 * 128], BF16, tag="ps")
                for t in range(BT):
                    nc.tensor.transpose(xps[:, t * 128:(t + 1) * 128],
                                        xhat[:, t, :], ident[:])
                nc.vector.tensor_copy(
                    xT[:], xps[:, 0:BT * 128].rearrange("p (t d) -> p t d", t=BT))

            if stage >= 3:
                psq = bigp.tile([128, NT, 128], BF16)
                pn2 = bigp.tile([128, NT], F32)
                pscale3 = bigp.tile([128, NT], F32)
                pbf = bigp.tile([128, NT, 128], BF16)
                pT = bigp.tile([128, NT, 128], BF16)   # [d, t, c%128]
                NG = 7
                partials = bigp.tile([128, BT * NG], F32)
                pTf = pT[:].rearrange("p t c -> p (t c)")

                def issue_group(bt, g):
                    w = 4 if g < 6 else 1
                    sp = psp.tile([128, 2048], F32, tag="ps")
                    for j in range(w):
                        ch = g * 4 + j
                        nc.tensor.matmul(
                            sp[:, j * 512:(j + 1) * 512],
                            xT[:, bt, :],
                            pTf[:, ch * 512:(ch + 1) * 512],
                            start=True, stop=True)
                    ej = pool.tile([128, 2048], BF16, tag="ejunk")
                    nc.scalar.activation(
                        ej[:, 0:w * 512], sp[:, 0:w * 512], AF.Exp,
                        bias=bias18[:, 0:1], scale=2.0,
                        accum_out=partials[:, bt * NG + g:bt * NG + g + 1])

                # groups issued once their pT chunks are drained
                NGR = BT * NG
                issued = set()
                clo = 0
                for i, ck in enumerate(CHUNKS):
                    lo, hi = clo, clo + ck
                    clo += ck
                    nc.gpsimd.tensor_tensor(psq[:, lo:hi, :],
                                            praw[:, lo:hi, :],
                                            praw[:, lo:hi, :], OP.mult)
                    nc.vector.tensor_reduce(pn2[:, lo:hi], psq[:, lo:hi, :],
                                            axis=AX.X, op=OP.add)
                    _rsqrt_dve(nc, pool, F32, OP, pscale3[:, lo:hi],
                               pn2[:, lo:hi], ck, scale=3.0)
                    if stage >= 4:
                        for g0 in range(lo, hi, 8):
                            w = min(8, hi - g0)
                            nc.vector.tensor_tensor(
                                pbf[:, g0:g0 + w, :], praw[:, g0:g0 + w, :],
                                pscale3[:, g0:g0 + w, None].to_broadcast(
                                    (128, w, 128)),
                                OP.mult)
                            tp = psp.tile([128, 8 * 128], BF16, tag="ps")
                            for j in range(w):
                                nc.tensor.transpose(
                                    tp[:, j * 128:(j + 1) * 128],
                                    pbf[:, g0 + j, :], ident[:])
                            nc.vector.tensor_copy(
                                pT[:, g0:g0 + w, :],
                                tp[:, 0:w * 128].rearrange(
                                    "p (t d) -> p t d", t=w))
                    if stage >= 5:
                        done_ch = (clo * 128) // 512   # complete 512-chunks
                        for g in range(NG):
                            need = (g * 4 + 4) if g < 6 else 25
                            if need <= done_ch and (0, g) not in issued:
                                for bt in range(BT):
                                    issued.add((bt, g))
                                    issue_group(bt, g)

            if stage >= 5:
                s_loc = bigp.tile([128, BT], F32)
                nc.vector.tensor_reduce(
                    s_loc[:], partials[:].rearrange("p (t g) -> p t g", t=BT),
                    axis=AX.X, op=OP.add)

            if stage >= 6:
                ag_in = dram.tile([128, BT], F32)
                ag_out = dram.tile([128 * N_CORES, BT], F32)
                nc.sync.dma_start(ag_in[:], s_loc[:])
                nc.gpsimd.collective_compute(
                    "AllGather", OP.bypass,
                    replica_groups=[list(range(N_CORES))],
                    ins=[ag_in.opt()], outs=[ag_out.opt()],
                )
                gath = bigp.tile([128, BT, N_CORES], F32)
                nc.sync.dma_start(gath[:],
                                  ag_out.rearrange("(r p) f -> p f r", p=128))
                s_tot = bigp.tile([128, BT], F32)
                nc.vector.tensor_reduce(s_tot[:], gath[:], axis=AX.X,
                                        op=OP.add)

            if stage >= 7:
                npos = pool.tile([128, BT], F32, tag="fin")
                nc.scalar.activation(npos[:], posd[:], AF.Exp, scale=-1.0)
                s1 = pool.tile([128, BT], F32, tag="fin")
                nc.vector.tensor_scalar(s1[:], s_tot[:], -float(PAD_CORR),
                                        None, OP.add)
                nc.vector.tensor_tensor(s1[:], s1[:], npos[:], OP.subtract)
                lse = pool.tile([128, BT], F32, tag="fin")
                nc.scalar.activation(lse[:], s1[:], AF.Ln)
                perb = pool.tile([128, BT], F32, tag="fin")
                nc.vector.tensor_tensor(perb[:], posd[:], lse[:], OP.add)
                csum = pool.tile([128, 1], F32, tag="fin")
                nc.vector.tensor_reduce(csum[:], perb[:], axis=AX.X, op=OP.add)
                nc.vector.tensor_tensor(csum[0:1, 0:1], csum[0:1, 0:1],
                                        dagj[:], OP.add)
                ones = pool.tile([128, 1], F32, tag="fin")
                nc.vector.memset(ones[:], 1.0)
                lps = psp.tile([1, 1], F32, tag="ps")
                nc.tensor.matmul(lps[:], ones[:], csum[:], start=True,
                                 stop=True)
                res = pool.tile([1, 1], F32, tag="fin")
                nc.scalar.activation(res[:], lps[:], AF.Copy, scale=1.0 / B)
                nc.sync.dma_start(out_ext[:], res[:])
            elif stage == 1:
                nc.sync.dma_start(out_ext[:], posd[0:1, 0:1])
            elif stage == 2:
                nc.gpsimd.dma_start(out_ext[:], xT[0:1, 0, 0:1])
            elif stage == 3:
                nc.sync.dma_start(out_ext[:], pscale3[0:1, 0:1])
            elif stage == 4:
                nc.gpsimd.dma_start(out_ext[:], pT[0:1, 0, 0:1])
            elif stage == 5:
                nc.sync.dma_start(out_ext[:], s_loc[0:1, 0:1])
            elif stage == 6:
                nc.sync.dma_start(out_ext[:], s_tot[0:1, 0:1])

    nc.compile()
    return nc


def make_in_maps(batch, labels, proxies):
    batch = np.ascontiguousarray(batch, dtype=np.float32)
    labels = np.asarray(labels).astype(np.int64)
    proxies = np.ascontiguousarray(proxies, dtype=np.float32)
    psel = np.ascontiguousarray(proxies[labels])        # indexing only
    ident = np.eye(128, dtype=np.float32).astype(ml_dtypes.bfloat16)
    ppad = np.zeros((N_CORES * SH, D), dtype=np.float32)
    ppad[:C] = proxies
    in_maps = []
    for i in range(N_CORES):
        in_maps.append({
            "pshard": np.ascontiguousarray(ppad[i * SH:(i + 1) * SH]),
            "batch": batch,
            "psel": psel,
            "ident": ident,
        })
    return in_maps


def _get_nc():
    if "nc" not in _CACHE:
        _CACHE["nc"] = build_graph()
    return _CACHE["nc"]


def kernel(batch, labels, proxies):
    nc = _get_nc()
    in_maps = make_in_maps(batch, labels, proxies)
    res = run_bass_kernel_spmd(nc, in_maps, core_ids=list(range(N_CORES)))
    return np.float32(res.results[0]["out"][0, 0])


if __name__ == "__main__":
    rng = np.random.default_rng(0)
    batch = rng.standard_normal((B, D)).astype(np.float32)
    labels = rng.integers(0, C, B).astype(np.int64)
    proxies = (rng.standard_normal((C, D)).astype(np.float32) / 8)
    out = kernel(batch=batch, labels=labels, proxies=proxies)
    print("loss:", out)
